# revision 14
# baseline (speedup 1.0000x reference)
"""Trainium2 Bass kernel for single-head attention with shared q/k/v projection.

Problem: B=8, S=2048, D=1024 single-head attention where q, k, v are all
projected with the same Dense(Wq, bq):
    q = query @ Wq + bq ; k = key @ Wq + bq ; v = value @ Wq + bq
    out = softmax(q k^T / sqrt(D)) @ v

Sharding: data-parallel over batch — one batch element per NeuronCore,
8 cores, no collectives. Each core runs an identical NEFF on its own
batch slice (SPMD via run_bass_kernel_spmd).

Per-core kernel (all matmuls in bf16, fp32 PSUM accumulate):
  1. query/key/value are cast fp32->bf16 in SBUF, staged to a DRAM scratch
     per 512-row block, and DMA-XBAR-transposed back as X^T [D, S] (D on
     partitions) — the PE contracts over the partition dim, so the
     D-contraction needs D-major operands.
  2. kT/qT = (W as stationary) products -> [D, S] layout; v is produced in
     natural [S, D] layout (value^T as stationary). Bias added during the
     PSUM->SBUF copy (per-partition for qT/kT, partition-broadcast tile
     for v).
  3. logits^T tiles [sk=128, sq<=512] = kT-chunk^T @ qT-chunk, accumulated
     over 8 d-chunks in PSUM; ScalarE computes E = exp(logits/32) straight
     out of PSUM into bf16 SBUF (no running max: logits are O(+-10), exp
     fits fp32 comfortably).
  4. out-chunk [sq=128, D] = sum_sk E^T-chunk @ v-chunk, with one extra
     N=1 matmul per chunk against a ones-column producing sum_sk(E) in the
     same accumulation pass (softmax normalizer for free).
  5. VectorE: out = psum * (1/sumexp) + bq, DMA to HBM.

Emission order interleaves attention with input prep so the prep DMA for
value (and late query blocks) hides under QK matmuls: W, key, query,
QK(0), QK(1), value, PV(0), QK(2), PV(1), QK(3), PV(2), PV(3).
"""

import os
import sys

import numpy as np

for _p in ("/opt/trn_rl_repo", "/root/.axon_site/_ro/trn_rl_repo"):
    if os.path.isdir(_p) and _p not in sys.path:
        sys.path.append(_p)

_B, _S, _D = 8, 2048, 1024
_NCORES = 8
_P = 128
# SWDGE casting DMA (DRAM fp32 -> DRAM bf16 in one transfer) models ~2%
# faster but was not validated on hardware in-session; the staged
# DVE-cast path below is hardware-validated. Keep False.
_SWDGE_CAST = False

_cache = {}


def _build_nc(phases=("prep", "proj", "attn")):
    from contextlib import ExitStack

    import concourse.mybir as mybir
    import concourse.tile as tile
    from concourse import bacc

    dt = mybir.dt
    f32 = dt.float32
    bf16 = dt.bfloat16
    Exp = mybir.ActivationFunctionType.Exp
    add_op = mybir.AluOpType.add

    P = _P
    SQ = _S // 512        # 4   sq chunks of 512
    SK = _S // P          # 16  sk chunks of 128
    KD = _D // P          # 8   contraction (d_in) chunks of 128
    DO = _D // P          # 8   d_out chunks of 128
    DN = _D // 512        # 2   d_out chunks of 512

    nc = bacc.Bacc("TRN2", target_bir_lowering=False, debug=False,
                   num_devices=_NCORES)

    q_d = nc.dram_tensor("query", [_S, _D], f32, kind="ExternalInput").ap()
    k_d = nc.dram_tensor("key", [_S, _D], f32, kind="ExternalInput").ap()
    v_d = nc.dram_tensor("value", [_S, _D], f32, kind="ExternalInput").ap()
    w_d = nc.dram_tensor("Wq", [_D, _D], f32, kind="ExternalInput").ap()
    b_d = nc.dram_tensor("bq", [_D], f32, kind="ExternalInput").ap()
    o_d = nc.dram_tensor("out", [_S, _D], f32, kind="ExternalOutput").ap()

    with tile.TileContext(nc) as tc:
        with ExitStack() as ctx:
            const = ctx.enter_context(tc.tile_pool(name="const", bufs=1))
            psum = ctx.enter_context(tc.tile_pool(name="psum", bufs=8, space="PSUM"))
            dram = ctx.enter_context(tc.tile_pool(name="dram", bufs=3, space="DRAM"))

            # ---- persistent constants / products ----
            # per-partition bias for d_out-major layouts: bq_col[p, do] = bq[do*128+p]
            bq_col = const.tile([P, DO], f32)
            nc.sync.dma_start(bq_col[:], b_d.rearrange("(o p) -> p o", p=P))

            # partition-broadcast bias for s-major layouts: bq_bcast[p, n] = bq[n]
            bq_bcast = const.tile([P, _D], f32)
            nc.sync.dma_start(bq_bcast[:], b_d[None, :].to_broadcast([P, _D]))

            ones_col = const.tile([P, 1], bf16)
            nc.vector.memset(ones_col[:], 1.0)

            qT = const.tile([P, KD, _S], bf16)            # q^T: [d_out, s]
            kT = const.tile([P, KD, _S], bf16)            # k^T: [d_out, s]
            vN = const.tile([P, SK, _D], bf16)            # v:   [s, d_out]

            inv_sqrt_d = 1.0 / float(np.sqrt(_D))
            o_r = o_d.rearrange("(so p) d -> so p d", p=P)

            if "prep" not in phases:
                # timing-probe mode: initialize attention operands in place
                for t_ in (qT, kT, vN):
                    nc.vector.memset(t_[:], 0.25)

            prep_ctx = ctx.enter_context(ExitStack())
            wpool = prep_ctx.enter_context(tc.tile_pool(name="wpool", bufs=1))
            work = prep_ctx.enter_context(tc.tile_pool(name="work", bufs=3))
            xtp = prep_ctx.enter_context(tc.tile_pool(name="xtp", bufs=3))

            W_bf = wpool.tile([P, KD, _D], bf16)          # W[(kd p), n] -> [p, kd, n]
            w_r = w_d.rearrange("(kd p) n -> kd p n", p=P)
            for kd in range(KD):
                wf = work.tile([P, _D], f32, tag="xf32")
                nc.sync.dma_start(wf[:], w_r[kd])
                nc.vector.tensor_copy(W_bf[:, kd, :], wf[:])

            def prep_proj(src, ti):
                """Cast src to bf16 (via DRAM block staging + XBAR transpose)
                and project, one 512-row block at a time. ti: 0=query->qT,
                1=key->kT, 2=value->vN."""
                if "prep" not in phases:
                    return
                src_r = src.rearrange("(so p) d -> so p d", p=P)
                for blk in range(SQ):
                    with nc.named_scope(f"prep{ti}"):
                        xbf_dram = dram.tile([512, _D], bf16, tag="xbf")
                        if _SWDGE_CAST:
                            # fp32 -> bf16 cast inside the DMA (SWDGE),
                            # DRAM -> DRAM, no SBUF staging / DVE cast
                            nc.gpsimd.dma_start(
                                xbf_dram[:],
                                src[blk * 512:(blk + 1) * 512, :])
                        else:
                            for soi in range(4):
                                so = blk * 4 + soi
                                xf = work.tile([P, _D], f32, tag="xf32")
                                nc.sync.dma_start(xf[:], src_r[so])
                                xb = work.tile([P, _D], bf16, tag="xb16")
                                nc.vector.tensor_copy(xb[:], xf[:])
                                nc.sync.dma_start(
                                    xbf_dram[soi * P:(soi + 1) * P, :], xb[:])
                        xTb = xtp.tile([P, KD, 512], bf16, tag="xT")
                        for di in range(KD):
                            nc.sync.dma_start_transpose(
                                xTb[:, di, :],
                                xbf_dram[:, di * P:(di + 1) * P])

                    if "proj" not in phases:
                        continue
                    with nc.named_scope(f"proj{ti}"):
                        if ti < 2:
                            # qT/kT[do*128+p, s] = sum_kd W[kd,do]^T . xT[kd, s]
                            dest = qT if ti == 0 else kT
                            sq = blk
                            for do in range(DO):
                                ps = psum.tile([P, 512], f32, tag="ps")
                                for kd in range(KD):
                                    nc.tensor.matmul(
                                        ps[:],
                                        W_bf[:, kd, do * P:(do + 1) * P],
                                        xTb[:, kd, :],
                                        start=(kd == 0), stop=(kd == KD - 1))
                                nc.vector.tensor_tensor(
                                    dest[:, do, sq * 512:(sq + 1) * 512],
                                    ps[:],
                                    bq_col[:, do:do + 1].to_broadcast([P, 512]),
                                    add_op)
                        else:
                            # v[sk*128+p, n] = sum_kd xT[kd, sk]^T . W[kd, n]
                            for ski in range(4):
                                sk = blk * 4 + ski
                                for dn in range(DN):
                                    ps = psum.tile([P, 512], f32, tag="ps")
                                    for kd in range(KD):
                                        nc.tensor.matmul(
                                            ps[:],
                                            xTb[:, kd, ski * P:(ski + 1) * P],
                                            W_bf[:, kd, dn * 512:(dn + 1) * 512],
                                            start=(kd == 0), stop=(kd == KD - 1))
                                    nc.vector.tensor_tensor(
                                        vN[:, sk, dn * 512:(dn + 1) * 512],
                                        ps[:],
                                        bq_bcast[:, dn * 512:(dn + 1) * 512],
                                        add_op)

            # ---- attention phase pieces ----
            epool = ctx.enter_context(tc.tile_pool(name="epool", bufs=2))
            opool = ctx.enter_context(tc.tile_pool(name="opool", bufs=3))
            E_tiles = {}

            def qk(sqo):
                with nc.named_scope("qk"):
                    E = epool.tile([P, SK, 512], bf16, tag="E")
                    E_tiles[sqo] = E
                    for sk in range(SK):
                        ps = psum.tile([P, 512], f32, tag="ps")
                        for kd in range(KD):
                            nc.tensor.matmul(
                                ps[:],
                                kT[:, kd, sk * P:(sk + 1) * P],
                                qT[:, kd, sqo * 512:(sqo + 1) * 512],
                                start=(kd == 0), stop=(kd == KD - 1))
                        nc.scalar.activation(E[:, sk, :], ps[:], Exp,
                                             scale=inv_sqrt_d)

            def pv(sqo):
                E = E_tiles.pop(sqo)
                with nc.named_scope("pv"):
                    for sqi in range(4):
                        soff = sqi * P
                        po0 = psum.tile([P, 512], f32, tag="ps")
                        po1 = psum.tile([P, 512], f32, tag="ps")
                        pn = psum.tile([P, 512], f32, tag="ps")
                        for sk in range(SK):
                            lhsT = E[:, sk, soff:soff + P]
                            st, sp = (sk == 0), (sk == SK - 1)
                            nc.tensor.matmul(po0[:], lhsT, vN[:, sk, 0:512],
                                             start=st, stop=sp)
                            nc.tensor.matmul(po1[:], lhsT, vN[:, sk, 512:1024],
                                             start=st, stop=sp)
                            nc.tensor.matmul(pn[:, 0:1], lhsT, ones_col[:],
                                             start=st, stop=sp)
                        rec = opool.tile([P, 1], f32, tag="rec")
                        nc.vector.reciprocal(rec[:], pn[:, 0:1])
                        ot = opool.tile([P, _D], f32, tag="osb")
                        nc.vector.tensor_scalar_mul(ot[:, 0:512], po0[:], rec[:])
                        nc.vector.tensor_scalar_mul(ot[:, 512:1024], po1[:], rec[:])
                        nc.vector.tensor_add(ot[:], ot[:], bq_bcast[:])
                        nc.sync.dma_start(o_r[sqo * 4 + sqi], ot[:])

            # ---- emission order: overlap value prep under attention ----
            attn = "attn" in phases
            prep_proj(k_d, 1)
            prep_proj(q_d, 0)
            if attn:
                qk(0)
                qk(1)
            prep_proj(v_d, 2)
            if attn:
                pv(0)
                qk(2)
                pv(1)
                qk(3)
                pv(2)
                pv(3)

    nc.compile()
    return nc


def _get_nc():
    if "nc" not in _cache:
        _cache["nc"] = _build_nc()
    return _cache["nc"]


def kernel(query, key, value, Wq, bq):
    from concourse.bass_utils import run_bass_kernel_spmd

    nc = _get_nc()
    Wq = np.ascontiguousarray(Wq, dtype=np.float32)
    bq = np.ascontiguousarray(bq, dtype=np.float32)
    in_maps = [
        {
            "query": np.ascontiguousarray(query[i], dtype=np.float32),
            "key": np.ascontiguousarray(key[i], dtype=np.float32),
            "value": np.ascontiguousarray(value[i], dtype=np.float32),
            "Wq": Wq,
            "bq": bq,
        }
        for i in range(_B)
    ]
    res = run_bass_kernel_spmd(nc, in_maps, core_ids=list(range(_NCORES)))
    return np.stack([res.results[i]["out"] for i in range(_B)], axis=0)
